# revision 6
# baseline (speedup 1.0000x reference)
import sys

sys.path.insert(0, "/opt/trn_rl_repo")

import numpy as np
import ml_dtypes

import concourse.bass as bass
import concourse.tile as tile
from concourse import mybir
from concourse import bass_utils
from concourse import bacc

B, T, S = 4, 2048, 2048
DQ, DKV, H, HD = 512, 1024, 8, 64
N_CORES = 8
TC = T // 2  # t-rows handled per core (batch b = c//2, t-half = c%2)

BF16 = ml_dtypes.bfloat16

_prog_cache = {}


def _build_program():
    f32 = mybir.dt.float32
    bf16 = mybir.dt.bfloat16
    FP = mybir.ActivationFunctionType

    nc = bacc.Bacc("TRN2", target_bir_lowering=False, debug=False,
                   num_devices=N_CORES)

    qT = nc.dram_tensor("qT", [DQ, TC], bf16, kind="ExternalInput").ap()
    kT = nc.dram_tensor("kT", [DKV, S], bf16, kind="ExternalInput").ap()
    vT = nc.dram_tensor("vT", [DKV, S], bf16, kind="ExternalInput").ap()
    wq = nc.dram_tensor("wq", [DQ, DQ], bf16, kind="ExternalInput").ap()
    wk = nc.dram_tensor("wk", [DKV, DQ], bf16, kind="ExternalInput").ap()
    wv = nc.dram_tensor("wv", [DKV, DQ], bf16, kind="ExternalInput").ap()
    wo = nc.dram_tensor("wo", [DQ, DQ], bf16, kind="ExternalInput").ap()
    attn_st = nc.dram_tensor("attn_st", [H, S, TC], f32, kind="ExternalOutput").ap()
    out = nc.dram_tensor("out", [TC, DQ], f32, kind="ExternalOutput").ap()

    with tile.TileContext(nc) as tc:
        with (
            tc.tile_pool(name="wpool", bufs=1) as wpool,
            tc.tile_pool(name="persist", bufs=1) as ppool,
            tc.tile_pool(name="psum", bufs=2, space="PSUM") as pspool,
            tc.tile_pool(name="psum_pv", bufs=2, space="PSUM") as pvpool,
        ):
            # ---- load weights ----
            wq_sb = wpool.tile([128, 4, DQ], bf16, tag="wq")
            nc.sync.dma_start(out=wq_sb[:], in_=wq.rearrange("(a p) n -> p a n", p=128))
            wk_sb = wpool.tile([128, 8, DQ], bf16, tag="wk")
            nc.sync.dma_start(out=wk_sb[:], in_=wk.rearrange("(a p) n -> p a n", p=128))
            wv_sb = wpool.tile([128, 8, DQ], bf16, tag="wv")
            nc.sync.dma_start(out=wv_sb[:], in_=wv.rearrange("(a p) n -> p a n", p=128))
            # wo laid out head-major on partitions 0-63: [hd=64, h=8, n=512]
            wo_sb = wpool.tile([64, 8, DQ], bf16, tag="wo")
            nc.sync.dma_start(out=wo_sb[:], in_=wo.rearrange("(h d) n -> d h n", d=64))
            ones_sb = wpool.tile([128, 128], f32, tag="ones")
            nc.vector.memset(ones_sb[:], 1.0)

            # ---- persistent intermediates ----
            QT_sb = ppool.tile([128, 4, TC], bf16, tag="qt")     # Q^T  [dq, t]
            KT_sb = ppool.tile([128, 4, S], bf16, tag="kt")      # K^T  [dq, s]
            V_sb = ppool.tile([128, 16, H, HD + 1], bf16, tag="v")  # V [s,h,hd+1]
            outT_sc = ppool.tile([64, H, TC], bf16, tag="ot")    # out^T per head

            nc.vector.memset(V_sb[:, :, :, HD:HD + 1], 1.0)

            # ---- P1: projections ----
            with tc.tile_pool(name="big", bufs=1) as bigpool:
                qT_sb = bigpool.tile([128, 4, TC], bf16, tag="qt")
                nc.sync.dma_start(out=qT_sb[:],
                                  in_=qT.rearrange("(a p) n -> p a n", p=128))
                kT_view = kT.rearrange("(a p) n -> p a n", p=128)
                kt0 = bigpool.tile([128, 4, S], bf16, tag="kt0")
                nc.sync.dma_start(out=kt0[:], in_=kT_view[:, 0:4, :])
                kt1 = bigpool.tile([128, 4, S], bf16, tag="kt1")
                nc.sync.dma_start(out=kt1[:], in_=kT_view[:, 4:8, :])
                vT_view = vT.rearrange("(a p) n -> p a n", p=128)
                vt0 = bigpool.tile([128, 4, S], bf16, tag="vt0")
                nc.sync.dma_start(out=vt0[:], in_=vT_view[:, 0:4, :])
                vt1 = bigpool.tile([128, 4, S], bf16, tag="vt1")
                nc.sync.dma_start(out=vt1[:], in_=vT_view[:, 4:8, :])

                # QT = Wq'^T @ qT   (Wq' pre-scaled by 1/sqrt(HD) on host)
                for m in range(4):
                    ps = pspool.tile([128, TC], f32, tag="ps")
                    for nh in range(2):
                        for k in range(4):
                            nc.tensor.matmul(
                                ps[:, nh * 512:(nh + 1) * 512],
                                lhsT=wq_sb[:, k, m * 128:(m + 1) * 128],
                                rhs=qT_sb[:, k, nh * 512:(nh + 1) * 512],
                                start=(k == 0), stop=(k == 3),
                            )
                    nc.scalar.activation(QT_sb[:, m, :], ps[:], FP.Copy)

                # KT = Wk^T @ kT
                for m in range(4):
                    for half in range(2):
                        ps = pspool.tile([128, TC], f32, tag="ps")
                        for nh in range(2):
                            nq = half * 2 + nh
                            for k in range(8):
                                kt = kt0 if k < 4 else kt1
                                nc.tensor.matmul(
                                    ps[:, nh * 512:(nh + 1) * 512],
                                    lhsT=wk_sb[:, k, m * 128:(m + 1) * 128],
                                    rhs=kt[:, k % 4, nq * 512:(nq + 1) * 512],
                                    start=(k == 0), stop=(k == 7),
                                )
                        nc.scalar.activation(
                            KT_sb[:, m, half * 1024:(half + 1) * 1024], ps[:], FP.Copy)

                # V = value @ Wv, stored [s_block, h, hd] with ones col at hd=64
                for st in range(16):
                    ps = pspool.tile([128, DQ], f32, tag="ps")
                    for k in range(8):
                        vt = vt0 if k < 4 else vt1
                        nc.tensor.matmul(
                            ps[:],
                            lhsT=vt[:, k % 4, st * 128:(st + 1) * 128],
                            rhs=wv_sb[:, k, :],
                            start=(k == 0), stop=(k == 7),
                        )
                    nc.scalar.activation(
                        V_sb[:, st, :, 0:HD],
                        ps[:].rearrange("p (h d) -> p h d", d=HD),
                        FP.Copy,
                    )

            # ---- P3: attention per head ----
            with (
                tc.tile_pool(name="a_uT", bufs=20) as apool,
                tc.tile_pool(name="attn_stage", bufs=4) as stpool,
                tc.tile_pool(name="outT_u", bufs=2) as oupool,
                tc.tile_pool(name="bcast", bufs=2) as bcpool,
                tc.tile_pool(name="out_stage", bufs=2) as ostpool,
            ):
                for h in range(8):
                    ch, po = h // 2, (h % 2) * 64
                    a_tiles = []
                    ps_pv = pvpool.tile([HD + 1, TC], f32, tag="pv")
                    for st in range(16):
                        ps_sc = pspool.tile([128, TC], f32, tag="ps")
                        for nh in range(2):
                            nc.tensor.matmul(
                                ps_sc[:, nh * 512:(nh + 1) * 512],
                                lhsT=KT_sb[po:po + 64, ch, st * 128:(st + 1) * 128],
                                rhs=QT_sb[po:po + 64, ch, nh * 512:(nh + 1) * 512],
                                start=True, stop=True,
                            )
                        a = apool.tile([128, TC], bf16)
                        nc.scalar.activation(a[:], ps_sc[:], FP.Exp)
                        a_tiles.append(a)
                        for nh in range(2):
                            nc.tensor.matmul(
                                ps_pv[:, nh * 512:(nh + 1) * 512],
                                lhsT=V_sb[:, st, h, :],
                                rhs=a[:, nh * 512:(nh + 1) * 512],
                                start=(st == 0), stop=(st == 15),
                            )
                    # phase 2: normalize + emit
                    outT_u = oupool.tile([HD + 1, TC], f32)
                    nc.scalar.activation(outT_u[:], ps_pv[:], FP.Copy)
                    # broadcast denom row (partition 64) to all 128 partitions
                    # via ones outer-product matmul, then reciprocal
                    ps_bc = pspool.tile([128, TC], f32, tag="ps")
                    for nh in range(2):
                        nc.tensor.matmul(
                            ps_bc[:, nh * 512:(nh + 1) * 512],
                            lhsT=ones_sb[HD:HD + 1, :],
                            rhs=outT_u[HD:HD + 1, nh * 512:(nh + 1) * 512],
                            start=True, stop=True,
                        )
                    bc = bcpool.tile([128, TC], f32)
                    nc.vector.reciprocal(bc[:], ps_bc[:])
                    for st in range(16):
                        at = stpool.tile([128, TC], f32)
                        nc.vector.tensor_tensor(
                            at[:], a_tiles[st][:], bc[:], mybir.AluOpType.mult)
                        nc.sync.dma_start(
                            out=attn_st[h, st * 128:(st + 1) * 128, :], in_=at[:])
                    nc.vector.tensor_tensor(
                        outT_sc[:, h, :], outT_u[0:HD, :], bc[0:HD, :],
                        mybir.AluOpType.mult)

                # ---- P4: out = concat_h(out_h) @ Wo ----
                for tt in range(8):
                    ps_o = pspool.tile([128, DQ], f32, tag="ps")
                    for h in range(8):
                        nc.tensor.matmul(
                            ps_o[:],
                            lhsT=outT_sc[:, h, tt * 128:(tt + 1) * 128],
                            rhs=wo_sb[:, h, :],
                            start=(h == 0), stop=(h == 7),
                        )
                    ot = ostpool.tile([128, DQ], f32)
                    nc.scalar.activation(ot[:], ps_o[:], FP.Copy)
                    nc.sync.dma_start(out=out[tt * 128:(tt + 1) * 128, :], in_=ot[:])

    nc.compile()
    return nc


def _fallback(query, key, value, key_padding_mask, attn_mask,
              Wq, bq, Wk, bk, Wv, bv, Wo, bo):
    scale = np.float32(np.sqrt(HD))
    Q = (query @ Wq + bq).reshape(B, T, H, HD)
    K = (key @ Wk + bk).reshape(B, S, H, HD)
    V = (value @ Wv + bv).reshape(B, S, H, HD)
    scores = np.einsum("bthd,bshd->bhts", Q, K) / scale
    scores = scores + attn_mask[None, None, :, :]
    pad = (key_padding_mask == 0)[:, None, None, :]
    scores = np.where(pad, -np.inf, scores)
    scores = scores - scores.max(axis=-1, keepdims=True)
    e = np.exp(scores)
    attn = e / e.sum(axis=-1, keepdims=True)
    out = np.einsum("bhts,bshd->bthd", attn, V).reshape(B, T, DQ)
    out = out @ Wo + bo
    return out.astype(np.float32), attn.astype(np.float32)


def kernel(query, key, value, key_padding_mask, attn_mask,
           Wq, bq, Wk, bk, Wv, bv, Wo, bo):
    query = np.asarray(query, np.float32)
    key = np.asarray(key, np.float32)
    value = np.asarray(value, np.float32)
    key_padding_mask = np.asarray(key_padding_mask)
    attn_mask = np.asarray(attn_mask, np.float32)
    Wq, Wk, Wv, Wo = (np.asarray(w, np.float32) for w in (Wq, Wk, Wv, Wo))
    bq, bk, bv, bo = (np.asarray(b, np.float32) for b in (bq, bk, bv, bo))

    fast = (
        not attn_mask.any()
        and np.all(key_padding_mask != 0)
        and not bq.any() and not bk.any() and not bv.any() and not bo.any()
    )
    if not fast:
        return _fallback(query, key, value, key_padding_mask, attn_mask,
                         Wq, bq, Wk, bk, Wv, bv, Wo, bo)

    if "nc" not in _prog_cache:
        _prog_cache["nc"] = _build_program()
    nc = _prog_cache["nc"]

    wq_b = (Wq / np.float32(np.sqrt(HD))).astype(BF16)
    wk_b = Wk.astype(BF16)
    wv_b = Wv.astype(BF16)
    wo_b = Wo.astype(BF16)

    in_maps = []
    for c in range(N_CORES):
        b, th = c // 2, c % 2
        in_maps.append({
            "qT": np.ascontiguousarray(
                query[b, th * TC:(th + 1) * TC, :].T).astype(BF16),
            "kT": np.ascontiguousarray(key[b].T).astype(BF16),
            "vT": np.ascontiguousarray(value[b].T).astype(BF16),
            "wq": wq_b, "wk": wk_b, "wv": wv_b, "wo": wo_b,
        })

    res = bass_utils.run_bass_kernel_spmd(nc, in_maps, core_ids=list(range(N_CORES)))

    out_full = np.empty((B, T, DQ), np.float32)
    attn_full = np.empty((B, H, T, S), np.float32)
    for c in range(N_CORES):
        b, th = c // 2, c % 2
        out_full[b, th * TC:(th + 1) * TC, :] = res.results[c]["out"]
        attn_full[b, :, th * TC:(th + 1) * TC, :] = \
            res.results[c]["attn_st"].transpose(0, 2, 1)
    return out_full, attn_full
